# revision 46
# baseline (speedup 1.0000x reference)
"""Multi-head attention (b=4, n=2048, dim=1024, 16 heads x 64) on 8 Trainium2
NeuronCores.

Sharding: data-parallel over batch (4) x tensor-parallel over head-groups (2).
Each core gets one batch element and 8 heads: it computes its slice of the QKV
projection, full attention for its heads, and a partial output projection.
The host sums the two head-group partials per batch element and adds b_out.

Per-core pipeline (fp32 data; matmul-feeding tiles float32r):
  A:  per 512-wide n-chunk: PE-transpose x tiles into xT chunk tiles (SBUF),
      then qT = Wq^T x^T, kT = Wk^T x^T (kept transposed, [inner, n] in
      128-row strips) and v = x Wv (natural [n, inner], augmented with a ones
      column per head so the PV matmul also emits the softmax denominator).
  B:  loop i-blocks (ib) OUTER, heads INNER: S^T j-tiles =
      matmul(lhsT=k^T_h j-block, rhs=q^T_h i-block) (scores transposed,
      [j, i]); exp on ScalarE (1/sqrt(dh) folded into the activation scale);
      PV matmul accumulates O_aug^T = v_aug^T @ P^T in PSUM ([dh+1, i]; last
      row = denominator). Tail: reciprocal of the denominator row, broadcast
      across partitions (gpsimd partition_broadcast or a K=1 matmul),
      multiply -> normalized O^T strip.
      After the 8 heads of an i-block finish, the output projection for that
      i-range runs immediately (fills PE bubbles left by the tails).
"""

import numpy as np

import concourse.bass as bass
import concourse.mybir as mybir
import concourse.tile as tile
from concourse import bacc, bass_utils
from concourse.masks import make_identity

F32 = mybir.dt.float32
AF = mybir.ActivationFunctionType

# Full-problem constants (hardcoded per the harness contract).
B_FULL, N_FULL, DIM_FULL = 4, 2048, 1024
HEADS_FULL, DH = 16, 64
N_CORES = 8
GROUPS = 2                       # head-group (tensor-parallel) factor
HPC = HEADS_FULL // GROUPS       # heads per core = 8
INNER_PC = HPC * DH              # per-core inner dim = 512

# Matmul compute dtype: float32r streams 1 row/cycle (vs 4 for float32) at
# slightly reduced precision. All tiles feeding matmuls carry this dtype
# (producers round into it); numpy float32 maps onto it unchanged.
MM_DT = mybir.dt.float32r


def ts(i, size):
    return slice(i * size, (i + 1) * size)


def emit_core_kernel(nc, tc, x, wqkv, wout, y, *, n, dim, hpc, dh,
                     mm_dt=MM_DT, ib=1024, bcast="gpsimd", phases="full"):
    inner = hpc * dh
    KC = dim // 128          # contraction chunks for the qkv projection
    S = inner // 128         # 128-row strips of the per-core inner dim
    JT = n // 128            # key/value j-tiles
    ib = min(ib, n)
    assert n % 512 == 0 and dim % 128 == 0 and inner % 128 == 0
    assert ib % 512 == 0 and n % ib == 0
    scale = float(1.0 / np.sqrt(dh))
    MD = mm_dt
    fc = min(512, dim)

    with (
        tc.tile_pool(name="const", bufs=1) as const_pool,
        tc.tile_pool(name="persist", bufs=1) as persist,
        tc.tile_pool(name="dram", bufs=1, space="DRAM") as dram_pool,
    ):
        qt_dram = dram_pool.tile([inner // 128, 128, n], mm_dt,
                                 name="qt_dram")
        # Memset/affine_select can only write plain fp32; constants that feed
        # matmuls are built in fp32 and rounded into mm_dt via DVE copies.
        ident = const_pool.tile([128, 128], F32, name="ident")
        make_identity(nc, ident)
        oneshc = const_pool.tile([128, hpc], F32, name="oneshc")
        nc.gpsimd.memset(oneshc, 1.0)
        if bcast == "matmul":
            ones_f32 = const_pool.tile([1, dh], F32, name="ones_f32")
            nc.gpsimd.memset(ones_f32, 1.0)
            ones_sb = const_pool.tile([1, dh], MD, name="ones_sb")
            nc.vector.tensor_copy(ones_sb, ones_f32)

        # Persistent SBUF tensors. qT is staged to DRAM (SBUF is tight) and
        # streamed back per (i-block, head) in phase B.
        kT, oT = [], []
        for s in range(S):
            kT.append(persist.tile([128, n], MD, name="kTs", tag=f"kT{s}"))
        v_sb = []
        for jt in range(JT):
            vt = persist.tile([128, hpc * (dh + 1)], MD, name="vts",
                              tag=f"v{jt}")
            v_sb.append(vt)
            nc.vector.tensor_copy(
                vt.rearrange("p (h c) -> p h c", c=dh + 1)[:, :, dh:dh + 1],
                oneshc.rearrange("p (h c) -> p h c", c=1))

        # ---- Phase A: x -> xT chunks (PE transpose) -> qT, kT, v_aug ----
        with (
            tc.tile_pool(name="a_w", bufs=1) as w_pool,
            tc.tile_pool(name="a_xin", bufs=4) as xin_pool,
            tc.tile_pool(name="a_xts", bufs=2) as xts_pool,
            tc.tile_pool(name="a_qstage", bufs=2) as qstage_pool,
            tc.tile_pool(name="a_psT", bufs=2, space="PSUM") as psT_pool,
            tc.tile_pool(name="a_ps", bufs=3, space="PSUM") as psA_pool,
        ):
            # First x tiles are on the critical path (first transposes);
            # emit their DMAs before the weight loads.
            first_x = []
            for j2 in range(4):
                x_in = xin_pool.tile([128, dim], F32, name="x_in")
                nc.sync.dma_start(x_in, x[ts(j2, 128), :])
                first_x.append(x_in)
            w_sb = []
            for kc in range(KC):
                wt = w_pool.tile([128, 3 * inner], MD, name="wt",
                                 tag=f"w{kc}")
                nc.sync.dma_start(wt, wqkv[ts(kc, 128), :])
                w_sb.append(wt)

            for nb in range(n // 512):
                # transpose the 4 x row-tiles of this chunk into xts
                xts = [xts_pool.tile([128, 512], MD, name="xts",
                                     tag=f"xts{kc}") for kc in range(KC)]
                for j2 in range(4):
                    it = nb * 4 + j2
                    if nb == 0:
                        x_in = first_x[j2]
                    else:
                        x_in = xin_pool.tile([128, dim], F32, name="x_in")
                        nc.sync.dma_start(x_in, x[ts(it, 128), :])
                    for kc in range(KC):
                        pt = psT_pool.tile([128, 128], F32, name="pt")
                        nc.tensor.transpose(pt, x_in[:, ts(kc, 128)], ident)
                        nc.vector.tensor_copy(xts[kc][:, ts(j2, 128)], pt)
                # qT / kT strips: out[m, i] over this 512-wide i chunk.
                # qT goes to DRAM (streamed back in phase B); kT stays in SBUF.
                for which in (0, 1):
                    for s in range(S):
                        ps = psA_pool.tile([128, 512], F32, name="psA")
                        base = which * inner + s * 128
                        for kc in range(KC):
                            nc.tensor.matmul(
                                ps, w_sb[kc][:, base:base + 128],
                                xts[kc],
                                start=(kc == 0), stop=(kc == KC - 1))
                        if which == 1:
                            nc.vector.tensor_copy(kT[s][:, ts(nb, 512)], ps)
                        else:
                            qs = qstage_pool.tile([128, 512], MD, name="qs")
                            nc.vector.tensor_copy(qs, ps)
                            nc.sync.dma_start(qt_dram[s, :, ts(nb, 512)], qs)
                # v natural: 4 row-tiles of 128 within this chunk.
                for j2 in range(4):
                    it = nb * 4 + j2
                    ps = psA_pool.tile([128, inner], F32, name="psAv",
                                       tag="psAv")
                    for kc in range(KC):
                        nc.tensor.matmul(
                            ps, xts[kc][:, ts(j2, 128)],
                            w_sb[kc][:, 2 * inner:3 * inner],
                            start=(kc == 0), stop=(kc == KC - 1))
                    nc.vector.tensor_copy(
                        v_sb[it].rearrange(
                            "p (h c) -> p h c", c=dh + 1)[:, :, 0:dh],
                        ps.rearrange("p (h c) -> p h c", c=dh))

        if phases == "a":
            nc.sync.dma_start(y[0:128, :].bitcast(mm_dt), kT[0][:, 0:dim])
            return

        # ---- Phase B+C: attention per i-block, projection interleaved ----
        for s in range(S):
            oT.append(persist.tile([128, n], MD, name="oTs", tag=f"oT{s}"))
        with (
            tc.tile_pool(name="c_w", bufs=1) as wout_pool,
            tc.tile_pool(name="b_psS", bufs=2, space="PSUM") as psS_pool,
            tc.tile_pool(name="b_psO", bufs=3, space="PSUM") as psO_pool,
            tc.tile_pool(name="b_pexp", bufs=3) as pexp_pool,
            tc.tile_pool(name="b_qst", bufs=4) as qst_pool,
            tc.tile_pool(name="b_tail", bufs=2) as tail_pool,
            tc.tile_pool(name="c_y", bufs=3) as y_pool,
            tc.tile_pool(name="c_ps", bufs=1, space="PSUM") as psC_pool,
        ):
            if bcast == "matmul":
                psB_pool = tc.alloc_tile_pool(name="b_psB", bufs=1,
                                              space="PSUM")
            wout_sb = []
            for t in range(S):
                wo = wout_pool.tile([128, dim], MD, name="wo", tag=f"wo{t}")
                nc.sync.dma_start(wo, wout[ts(t, 128), :])
                wout_sb.append(wo)

            def emit_tail(po_c, h, ibx, c):
                # normalize rows 0..dh-1 of one half-block by its denominator
                s_, r_ = divmod(h * dh, 128)
                recip_f = tail_pool.tile([1, 512], F32, name="recip_f")
                nc.vector.reciprocal(recip_f, po_c[dh:dh + 1, :])
                bc = tail_pool.tile([dh, 512], F32, name="bc")
                if bcast == "gpsimd":
                    nc.gpsimd.partition_broadcast(bc, recip_f)
                else:
                    recip = tail_pool.tile([1, 512], MD, name="recip")
                    nc.vector.tensor_copy(recip, recip_f)
                    pb = psB_pool.tile([dh, 512], F32, name="pb")
                    nc.tensor.matmul(pb, ones_sb, recip,
                                     start=True, stop=True)
                    nc.vector.tensor_copy(bc, pb)
                off = ibx * ib + c * 512
                nc.vector.tensor_mul(
                    oT[s_][r_:r_ + dh, off:off + 512], po_c[0:dh, :], bc)

            ysb_open = {}

            def emit_proj_group(it, c):
                # one PSUM-group slice of the projection for i-tile `it`
                if c == 0:
                    ysb_open[it] = y_pool.tile([128, dim], F32, name="ysb")
                ysb = ysb_open[it]
                ps = psC_pool.tile([128, fc], F32, name="psC")
                for t in range(S):
                    nc.tensor.matmul(
                        ps, oT[t][:, ts(it, 128)],
                        wout_sb[t][:, ts(c, fc)],
                        start=(t == 0), stop=(t == S - 1))
                nc.vector.tensor_copy(ysb[:, ts(c, fc)], ps)
                if c == dim // fc - 1:
                    nc.sync.dma_start(y[ts(it, 128), :], ysb)
                    del ysb_open[it]

            def emit_proj(it):
                for c in range(dim // fc):
                    emit_proj_group(it, c)

            # Flat software pipeline over (ibx, h, jt): PV matmuls lag one
            # step behind S/exp in the PE stream (the FIFO carries across
            # head boundaries, so ScalarE never runs dry), head tails fire
            # when their last PV pops, and the previous i-block's projection
            # tiles interleave one-per-head into the next block.
            pend = []          # (po, pexp, jt, h, ibx)

            def pop_pend():
                po, pexp, jt, h, ibx = pend.pop(0)
                vcol = slice(h * (dh + 1), (h + 1) * (dh + 1))
                for c in range(ib // 512):
                    nc.tensor.matmul(
                        po[c], v_sb[jt][:, vcol],
                        pexp[:, ts(c, 512)],
                        start=(jt == 0), stop=(jt == JT - 1))
                if jt == JT - 1:
                    for c in range(ib // 512):
                        emit_tail(po[c], h, ibx, c)

            n_ibx = n // ib
            itpb = ib // 128                     # i-tiles per block
            seq = [(bx, hh) for bx in range(n_ibx) for hh in range(hpc)]
            qst_tiles = {}

            def load_qst(i):
                # prefetch the q i-block slice for sequence position i
                if i < len(seq) and i not in qst_tiles:
                    bx, hh = seq[i]
                    s2, r2 = divmod(hh * dh, 128)
                    t = qst_pool.tile([128, ib], MD, name="qst")
                    nc.sync.dma_start(
                        t[r2:r2 + dh, :], qt_dram[s2, r2:r2 + dh, ts(bx, ib)])
                    qst_tiles[i] = t

            proj_due = []
            for ibx in range(n_ibx):
                for h in range(hpc):
                    gi = ibx * hpc + h
                    load_qst(gi)
                    load_qst(gi + 1)
                    load_qst(gi + 2)
                    s_, r_ = divmod(h * dh, 128)
                    kTh = kT[s_][r_:r_ + dh, :]
                    qTh = qst_tiles.pop(gi)[r_:r_ + dh, :]
                    po = [psO_pool.tile([dh + 1, 512], F32, name="po")
                          for _ in range(ib // 512)]
                    # spread the previous block's projection groups through
                    # the jt loop so they fill PE slack without starving ACT
                    spread = max(1, JT // max(1, -(-len(proj_due) // hpc) + 1))
                    for jt in range(JT):
                        psS = psS_pool.tile([128, ib], F32, name="psS")
                        for c in range(ib // 512):
                            nc.tensor.matmul(
                                psS[:, ts(c, 512)], kTh[:, ts(jt, 128)],
                                qTh[:, ts(c, 512)],
                                start=True, stop=True)
                        pexp = pexp_pool.tile([128, ib], MD, name="pexp")
                        nc.scalar.activation(pexp, psS, AF.Exp, scale=scale)
                        pend.append((po, pexp, jt, h, ibx))
                        while len(pend) > 1:
                            pop_pend()
                        if proj_due and jt % spread == spread - 1 and jt < JT - 1:
                            emit_proj_group(*proj_due.pop(0))
                while proj_due:   # leftovers from the previous block
                    emit_proj_group(*proj_due.pop(0))
                proj_due = [(it, c) for it in range(ibx * itpb,
                                                    (ibx + 1) * itpb)
                            for c in range(dim // fc)]
                if ibx == n_ibx - 1:
                    while pend:
                        pop_pend()
                    for it, c in proj_due:
                        emit_proj_group(it, c)
            if bcast == "matmul":
                psB_pool.release()


_BUILD_CACHE = {}


def build_nc(n=N_FULL, dim=DIM_FULL, hpc=HPC, dh=DH, mm_dt=MM_DT, ib=1024,
             bcast="gpsimd", phases="full"):
    key = (n, dim, hpc, dh, str(mm_dt), ib, bcast, phases)
    if key in _BUILD_CACHE:
        return _BUILD_CACHE[key]
    inner = hpc * dh
    nc = bacc.Bacc("TRN2", target_bir_lowering=False, debug=False)
    x = nc.dram_tensor("x", [n, dim], F32, kind="ExternalInput").ap()
    wqkv = nc.dram_tensor("w_qkv", [dim, 3 * inner], mm_dt,
                          kind="ExternalInput").ap()
    wout = nc.dram_tensor("w_out", [inner, dim], mm_dt,
                          kind="ExternalInput").ap()
    y = nc.dram_tensor("y", [n, dim], F32, kind="ExternalOutput").ap()
    with tile.TileContext(nc) as tc:
        with nc.allow_low_precision(
                reason="float32r is 4-byte; PSUM accumulation stays fp32"):
            emit_core_kernel(nc, tc, x, wqkv, wout, y, n=n, dim=dim, hpc=hpc,
                             dh=dh, mm_dt=mm_dt, ib=ib, bcast=bcast,
                             phases=phases)
    nc.compile()
    _BUILD_CACHE[key] = nc
    return nc


def make_in_maps(x, w_qkv, w_out):
    """Shard full inputs into the 8 per-core input maps."""
    x = np.asarray(x, dtype=np.float32)
    w_qkv = np.asarray(w_qkv, dtype=np.float32)
    w_out = np.asarray(w_out, dtype=np.float32)
    qk_off = HEADS_FULL * DH          # 1024: start of K block in w_qkv
    in_maps = []
    for c in range(N_CORES):
        b, g = divmod(c, GROUPS)
        cols = ts(g, INNER_PC)
        wq = w_qkv[:, cols]
        wk = w_qkv[:, qk_off + g * INNER_PC: qk_off + (g + 1) * INNER_PC]
        wv = w_qkv[:, 2 * qk_off + g * INNER_PC: 2 * qk_off + (g + 1) * INNER_PC]
        in_maps.append({
            "x": np.ascontiguousarray(x[b]),
            "w_qkv": np.ascontiguousarray(np.concatenate([wq, wk, wv], axis=1)),
            "w_out": np.ascontiguousarray(w_out[cols, :]),
        })
    return in_maps


def kernel(x, w_qkv, w_out, b_out, trace=False):
    b_out = np.asarray(b_out, dtype=np.float32)
    nc = build_nc()
    in_maps = make_in_maps(x, w_qkv, w_out)
    res = bass_utils.run_bass_kernel_spmd(
        nc, in_maps, core_ids=list(range(N_CORES)), trace=trace)
    ys = [r["y"] for r in res.results]
    out = np.empty((B_FULL, N_FULL, DIM_FULL), dtype=np.float32)
    for b in range(B_FULL):
        out[b] = ys[GROUPS * b] + ys[GROUPS * b + 1] + b_out[None, :]
    if trace:
        kernel.last_result = res
    return out
